# revision 47
# baseline (speedup 1.0000x reference)
"""Trainium2 Bass kernel for a dense fp32 MultiHeadAttention layer.

Problem (hardcoded): B=4, T=S=2048, C=1024, 16 heads x 64 dims, fp32.
  q = query @ Wq.T + bq ; k,v likewise
  scores = (q k^T) * D**-0.5 + attn_mask + padding_mask
  out = softmax(scores) @ v -> reshape -> @ Wout.T + bout

Sharding over 8 NeuronCores: core c = (batch b = c//2, head-group g = c%2).
Each core handles one batch and 8 of the 16 heads:
  - column-parallel q/k/v projections (512-dim slice of the projections)
  - attention for its 8 heads (full T x S, on-chip scores)
  - row-parallel out_proj producing a partial (T, C) output
Host sums the two partials per batch and adds the bias terms
(bout + bv @ Wout.T, which commutes with softmax since sum(weights)=1).

v2 layout/schedule (single fused phase, bf16 matmul operands):
  - all matmul operands are bf16 (halves SBUF/DMA, enables FWL weight
    loads); PSUM accumulation and softmax denominators stay fp32.
  - schedule is software-pipelined so the PE never idles and the ACT
    engine (exp) starts ~8us in instead of after a separate projection
    phase: qproj(t0) -> kproj chunks interleaved with scores(pr0) ->
    vproj chunks interleaved with scores(pr1) -> rolling PV/scores with
    qproj(t+1)/scores(t+1) hoisted before PV(pr3)/out-proj(t).
  - scores computed transposed (s on partitions): psc = kT.T @ qT, so
    softmax's s-reduction rides the PV matmul via a ones column in v.
  - normalization: both heads' denominator rows -> one [2,TCH] recip,
    broadcast across partitions with a single 2-row one-hot matmul
    (e2.T @ rrow), then one DVE multiply per head pair.
"""

import os
import numpy as np
import ml_dtypes

import concourse.bass as bass
import concourse.mybir as mybir
import concourse.tile as tile
from concourse import bacc
from concourse.bass_utils import run_bass_kernel_spmd

# ---- problem constants ----
B, T, S, C = 4, 2048, 2048, 1024
H, D = 16, 64
NCORES = 8
F = 512            # per-core projection slice (8 heads x 64)
SCALE = D ** -0.5
P = 128
TCH = 512          # t-chunk (score free dim)
NTC = T // TCH     # 4
NSC = S // P       # 16 s-chunks
NFC = F // P       # 4 f-chunks per core
NCC = C // P       # 8 contraction chunks
SW = 512           # s-window for k/v projection chunks
NSW = S // SW      # 4
HW = 65            # v width per head incl. ones column

FP32 = mybir.dt.float32
FP32R = mybir.dt.float32r
DT = mybir.dt.bfloat16
NP_DT = ml_dtypes.bfloat16

LAST_EXEC_NS = None
LAST_TRACE = None
LAST_NC = None
LAST_IN_MAPS = None


def build(use_mask: bool, debug: bool = False):
    nc = bacc.Bacc("TRN2", target_bir_lowering=False, debug=False,
                   num_devices=NCORES)

    xq = nc.dram_tensor("xq", [C, T], DT, kind="ExternalInput")
    xk = nc.dram_tensor("xk", [C, S], DT, kind="ExternalInput")
    xv = nc.dram_tensor("xv", [C, S], DT, kind="ExternalInput")
    wq = nc.dram_tensor("wq", [C, F], DT, kind="ExternalInput")
    wk = nc.dram_tensor("wk", [C, F], DT, kind="ExternalInput")
    wv = nc.dram_tensor("wv", [C, F], DT, kind="ExternalInput")
    wo = nc.dram_tensor("wo", [F, C], DT, kind="ExternalInput")
    bqr = nc.dram_tensor("bqr", [P, NFC], FP32, kind="ExternalInput")
    bkr = nc.dram_tensor("bkr", [P, NFC], FP32, kind="ExternalInput")
    if use_mask:
        emask = nc.dram_tensor("emask", [S, T], FP32, kind="ExternalInput")
    out = nc.dram_tensor("out", [T, C], DT, kind="ExternalOutput")
    if debug:
        dbg_q = nc.dram_tensor("dbg_q", [P, NFC, TCH], DT,
                               kind="ExternalOutput")
        dbg_kt = nc.dram_tensor("dbg_kt", [P, NFC, TCH], DT,
                                kind="ExternalOutput")
        dbg_v = nc.dram_tensor("dbg_v", [P, 2, 8 * HW], DT,
                               kind="ExternalOutput")
        dbg_exp = nc.dram_tensor("dbg_exp", [P, 2, 2 * TCH], DT,
                                 kind="ExternalOutput")
        dbg_rr = nc.dram_tensor("dbg_rr", [1, 2 * TCH], FP32,
                                kind="ExternalOutput")
        dbg_rrb = nc.dram_tensor("dbg_rrb", [1, 2 * TCH], FP32,
                                 kind="ExternalOutput")
        dbg_attn = nc.dram_tensor("dbg_attn", [P, NFC, TCH], DT,
                                  kind="ExternalOutput")

    xq_r = xq.rearrange("(cc p) t -> p cc t", p=P)
    xk_r = xk.rearrange("(cc p) s -> p cc s", p=P)
    xv_r = xv.rearrange("(cc p) s -> p cc s", p=P)
    wq_r = wq.rearrange("(cc p) f -> p cc f", p=P)
    wk_r = wk.rearrange("(cc p) f -> p cc f", p=P)
    wv_r = wv.rearrange("(cc p) f -> p cc f", p=P)
    wo_r = wo.rearrange("(dc p) f -> p dc f", p=P)

    with tile.TileContext(nc) as tc:
        with (
            tc.tile_pool(name="const", bufs=1) as cp,
            tc.tile_pool(name="xs", bufs=3) as xsp,        # xk/xv staging
            tc.tile_pool(name="xqs", bufs=2) as xqp,       # xq staging
            tc.tile_pool(name="expp", bufs=2) as ep,       # exp double buffer
            tc.tile_pool(name="work", bufs=4) as ws,       # small tiles
            tc.tile_pool(name="ob", bufs=4) as obp,        # out bounce
            tc.tile_pool(name="psc", bufs=2, space="PSUM") as pscp,
            tc.tile_pool(name="ppv", bufs=1, space="PSUM") as ppvp,
            tc.tile_pool(name="pg", bufs=2, space="PSUM") as pgp,
        ):
            wq_sb = cp.tile([P, NCC, F], DT, tag="wq")
            wk_sb = cp.tile([P, NCC, F], DT, tag="wk")
            wv_sb = cp.tile([P, NCC, F], DT, tag="wv")
            wo_sb = cp.tile([P, NFC, C], DT, tag="wo")
            bq_sb = cp.tile([P, NFC], FP32, tag="bq")
            bk_sb = cp.tile([P, NFC], FP32, tag="bk")
            onesb_sb = cp.tile([1, D], DT, tag="onesb")
            kT_sb = cp.tile([P, NFC, S], DT, tag="kT")
            v_sb = cp.tile([P, NSC, 8 * HW], DT, tag="v")
            qT_sb = cp.tile([P, NFC, TCH], DT, tag="qT")
            attnT = cp.tile([P, NFC, TCH], DT, tag="attnT")
            rr32_sb = cp.tile([1, 2 * TCH], FP32, tag="rr32")
            rr32b_sb = cp.tile([1, 2 * TCH], FP32, tag="rr32b")
            rrow2_sb = cp.tile([1, 2 * TCH], DT, tag="rrow2")

            # ---- startup DMAs: interleave q-path (gates the first matmul)
            # with k-path chunks so kproj inputs land during qproj compute
            xq_t0 = xqp.tile([P, NCC, TCH], DT, tag="xq", name="xq_t")
            xk_t0 = xsp.tile([P, NCC, SW], DT, tag="xs", name="xk_t")
            for cc in range(NCC):
                nc.sync.dma_start(wq_sb[:, cc, :], wq_r[:, cc, :])
                nc.sync.dma_start(xq_t0[:, cc, :], xq_r[:, cc, 0:TCH])
                nc.sync.dma_start(wk_sb[:, cc, :], wk_r[:, cc, :])
                nc.sync.dma_start(xk_t0[:, cc, :], xk_r[:, cc, 0:SW])
            nc.sync.dma_start(bq_sb[:], bqr[:])
            nc.sync.dma_start(bk_sb[:], bkr[:])

            # ---- constants: ones row (bcast stationary) + ones columns in v
            one_sb = cp.tile([P, 1], FP32, tag="one")
            nc.any.memset(one_sb[:], 1.0)
            nc.vector.tensor_copy(onesb_sb[:],
                                  one_sb[0:1, 0:1].to_broadcast((1, D)))
            ones_dst = v_sb[:].rearrange("p s (h e) -> p s h e", e=HW)[:, :, :, D]
            nc.vector.tensor_copy(ones_dst,
                                  one_sb[:, 0:1].to_broadcast(ones_dst.shape))

            def qproj(tcx, xq_t):
                t0 = tcx * TCH
                for fcp in range(2):
                    psq = [pgp.tile([P, TCH], FP32, tag="pg", name="psq")
                           for _ in range(2)]
                    for cc in range(NCC):
                        for i in range(2):
                            fc = fcp * 2 + i
                            nc.tensor.matmul(
                                psq[i][:],
                                wq_sb[:, cc, fc * P:(fc + 1) * P],
                                xq_t[:, cc, :],
                                start=(cc == 0), stop=(cc == NCC - 1))
                    for i in range(2):
                        fc = fcp * 2 + i
                        nc.vector.tensor_scalar_add(
                            qT_sb[:, fc, :], psq[i][:], bq_sb[:, fc:fc + 1])

            def kproj(sw, xk_t):
                s0 = sw * SW
                for fcp in range(2):
                    psk = [pgp.tile([P, SW], FP32, tag="pg", name="psk")
                           for _ in range(2)]
                    for cc in range(NCC):
                        for i in range(2):
                            fc = fcp * 2 + i
                            nc.tensor.matmul(
                                psk[i][:],
                                wk_sb[:, cc, fc * P:(fc + 1) * P],
                                xk_t[:, cc, :],
                                start=(cc == 0), stop=(cc == NCC - 1))
                    for i in range(2):
                        fc = fcp * 2 + i
                        nc.vector.tensor_scalar_add(
                            kT_sb[:, fc, s0:s0 + SW],
                            psk[i][:], bk_sb[:, fc:fc + 1])

            def vproj(sw, xv_t):
                for ssp in range(2):
                    psv = [pgp.tile([P, F], FP32, tag="pg", name="psv")
                           for _ in range(2)]
                    for cc in range(NCC):
                        for i in range(2):
                            ss = ssp * 2 + i
                            nc.tensor.matmul(
                                psv[i][:],
                                xv_t[:, cc, ss * P:(ss + 1) * P],
                                wv_sb[:, cc, :],
                                start=(cc == 0), stop=(cc == NCC - 1))
                    for i in range(2):
                        sc = sw * 4 + ssp * 2 + i
                        dst = v_sb[:, sc, :].rearrange(
                            "p (h e) -> p h e", e=HW)[:, :, 0:D]
                        src = psv[i][:].rearrange("p (h e) -> p h e", e=D)
                        nc.vector.tensor_copy(dst, src)

            def scores(tcx, pr, expT, sc_list):
                t0 = tcx * TCH
                for sc in sc_list:
                    psc = pscp.tile([P, 2, TCH], FP32, tag="psc", name="psc")
                    for h in range(2):
                        nc.tensor.matmul(
                            psc[:, h, :],
                            kT_sb[h * D:(h + 1) * D, pr, sc * P:(sc + 1) * P],
                            qT_sb[h * D:(h + 1) * D, pr, :],
                            start=True, stop=True)
                    nc.scalar.activation(
                        expT[:, sc, :], psc[:].rearrange("p a b -> p (a b)"),
                        mybir.ActivationFunctionType.Exp, scale=SCALE)
                    if use_mask:
                        em_t = ws.tile([P, TCH], FP32, tag="emk", name="em_t")
                        nc.sync.dma_start(
                            em_t[:], emask[sc * P:(sc + 1) * P, t0:t0 + TCH])
                        for h in range(2):
                            nc.vector.tensor_mul(
                                expT[:, sc, h * TCH:(h + 1) * TCH],
                                expT[:, sc, h * TCH:(h + 1) * TCH],
                                em_t[:])

            def pv(pr, expT, dump=False):
                """PV accumulation + denominator recip; returns pbc-deferred
                closure (bcast matmul + final multiply) to emit later."""
                ppv2 = ppvp.tile([HW, 2, TCH], FP32, tag="ppv", name="ppv")
                for sc in range(NSC):
                    for h in range(2):
                        hh = pr * 2 + h
                        nc.tensor.matmul(
                            ppv2[:, h, :],
                            v_sb[:, sc, hh * HW:(hh + 1) * HW],
                            expT[:, sc, h * TCH:(h + 1) * TCH],
                            start=(sc == 0), stop=(sc == NSC - 1))
                # denominators (PSUM row D, both heads contiguous across the
                # two banks) -> one copy to row 0 -> one recip at partition 0
                # (reciprocal_approx_fast misbehaves at partition offset 64)
                # -> one bf16 convert for the bcast matmul
                nc.vector.tensor_copy(
                    rr32_sb[:], ppv2[D:D + 1, :, :].rearrange("p a b -> p (a b)"))
                nc.vector.reciprocal_approx_fast(rr32b_sb[:], rr32_sb[:])
                nc.vector.tensor_copy(rrow2_sb[:], rr32b_sb[:])
                # numerators -> attnT (releases ppv banks)
                for h in range(2):
                    nc.vector.tensor_copy(attnT[h * D:(h + 1) * D, pr, :],
                                          ppv2[0:D, h, :])
                if dump:
                    nc.sync.dma_start(dbg_exp[:], expT[:, 0:2, :])
                    nc.sync.dma_start(dbg_rr[:], rr32_sb[:])
                    nc.sync.dma_start(dbg_rrb[:], rr32b_sb[:])

                def bcast():
                    pbc = pgp.tile([P, TCH], FP32, tag="pg", name="pbc")
                    for h in range(2):
                        nc.tensor.matmul(
                            pbc[h * D:(h + 1) * D, :],
                            onesb_sb[:],
                            rrow2_sb[0:1, h * TCH:(h + 1) * TCH],
                            start=True, stop=True)
                    nc.vector.tensor_mul(attnT[:, pr, :], attnT[:, pr, :],
                                         pbc[:])
                return bcast

            def outproj(tcx):
                t0 = tcx * TCH
                last = tcx == NTC - 1
                for tw in range(TCH // P):
                    for fh in range(2):
                        po = pgp.tile([P, TCH], FP32, tag="pg", name="po")
                        for dc in range(NFC):
                            nc.tensor.matmul(
                                po[:],
                                attnT[:, dc, tw * P:(tw + 1) * P],
                                wo_sb[:, dc, fh * TCH:(fh + 1) * TCH],
                                start=(dc == 0), stop=(dc == NFC - 1))
                        ob = obp.tile([P, TCH], DT, tag="ob", name="ob")
                        # in the drain the exp stream is over: split the
                        # bounce copies across ACT and DVE to shorten it
                        if last and (tw * 2 + fh) % 2 == 1:
                            nc.scalar.copy(ob[:], po[:])
                        else:
                            nc.vector.tensor_copy(ob[:], po[:])
                        nc.sync.dma_start(
                            out[t0 + tw * P: t0 + (tw + 1) * P,
                                fh * TCH:(fh + 1) * TCH],
                            ob[:])

            # ---------------- prologue: tcx=0 ----------------
            qproj(0, xq_t0)

            exp_bufs = {}

            def new_expT(tcx, pr):
                t_ = ep.tile([P, NSC, 2 * TCH], DT, tag="expT", name="expT")
                exp_bufs[(tcx, pr)] = t_
                return t_

            # k-projection interleaved with scores(0, pr0)
            e00 = new_expT(0, 0)
            for sw in range(NSW):
                if sw == 0:
                    xk_t = xk_t0
                else:
                    xk_t = xsp.tile([P, NCC, SW], DT, tag="xs", name="xk_t")
                    for cc in range(NCC):
                        nc.sync.dma_start(xk_t[:, cc, :],
                                          xk_r[:, cc, sw * SW:(sw + 1) * SW])
                kproj(sw, xk_t)
                scores(0, 0, e00, range(4 * sw, 4 * sw + 4))
                if sw == 0:
                    for cc in range(NCC):
                        nc.sync.dma_start(wv_sb[:, cc, :], wv_r[:, cc, :])

            # v-projection interleaved with scores(0, pr1)
            e01 = new_expT(0, 1)
            for sw in range(NSW):
                xv_t = xsp.tile([P, NCC, SW], DT, tag="xs", name="xv_t")
                for cc in range(NCC):
                    nc.sync.dma_start(xv_t[:, cc, :],
                                      xv_r[:, cc, sw * SW:(sw + 1) * SW])
                vproj(sw, xv_t)
                scores(0, 1, e01, range(4 * sw, 4 * sw + 4))
                if sw == 0:
                    for dc in range(NFC):
                        nc.sync.dma_start(wo_sb[:, dc, :], wo_r[:, dc, :])

            # ---------------- rolling main loop ----------------
            xq_tiles = {0: xq_t0}
            for tcx in range(NTC):
                if tcx + 1 < NTC:
                    xq_t = xqp.tile([P, NCC, TCH], DT, tag="xq", name="xq_t")
                    xq_tiles[tcx + 1] = xq_t
                    for cc in range(NCC):
                        nc.sync.dma_start(
                            xq_t[:, cc, :],
                            xq_r[:, cc, (tcx + 1) * TCH:(tcx + 2) * TCH])

                bc0 = pv(0, exp_bufs.pop((tcx, 0)), dump=(debug and tcx == 0))
                if debug and tcx == 0:
                    nc.sync.dma_start(dbg_q[:], qT_sb[:])
                    nc.sync.dma_start(dbg_kt[:], kT_sb[:, :, 0:TCH])
                    nc.sync.dma_start(dbg_v[:], v_sb[:, 0:2, :])
                scores(tcx, 2, new_expT(tcx, 2), range(NSC))
                bc0()
                bc1 = pv(1, exp_bufs.pop((tcx, 1)))
                scores(tcx, 3, new_expT(tcx, 3), range(NSC))
                bc1()
                bc2 = pv(2, exp_bufs.pop((tcx, 2)))
                if tcx + 1 < NTC:
                    qproj(tcx + 1, xq_tiles.pop(tcx + 1))
                    scores(tcx + 1, 0, new_expT(tcx + 1, 0), range(NSC))
                bc2()
                bc3 = pv(3, exp_bufs.pop((tcx, 3)))
                if tcx + 1 < NTC:
                    scores(tcx + 1, 1, new_expT(tcx + 1, 1), range(NSC))
                bc3()
                if debug and tcx == 0:
                    nc.sync.dma_start(dbg_attn[:], attnT[:])
                outproj(tcx)

    nc.compile()
    return nc


_CACHE = {}


def _get(use_mask: bool):
    dbg = bool(int(os.environ.get("MHA_DEBUG", "0")))
    key = (use_mask, dbg)
    if key not in _CACHE:
        _CACHE[key] = build(use_mask, debug=dbg)
    return _CACHE[key]


def kernel(query, key, value, attn_mask, key_padding_mask,
           Wq, bq, Wk, bk, Wv, bv, Wout, bout):
    global LAST_EXEC_NS, LAST_TRACE
    query = np.asarray(query, np.float32)
    key = np.asarray(key, np.float32)
    value = np.asarray(value, np.float32)
    attn_mask = np.asarray(attn_mask, np.float32)
    key_padding_mask = np.asarray(key_padding_mask)
    Wq, bq = np.asarray(Wq, np.float32), np.asarray(bq, np.float32)
    Wk, bk = np.asarray(Wk, np.float32), np.asarray(bk, np.float32)
    Wv, bv = np.asarray(Wv, np.float32), np.asarray(bv, np.float32)
    Wout, bout = np.asarray(Wout, np.float32), np.asarray(bout, np.float32)

    use_mask = bool(np.any(attn_mask)) or bool(np.any(key_padding_mask))
    nc = _get(use_mask)

    def cvt(a):
        return np.ascontiguousarray(a).astype(NP_DT)

    in_maps = []
    for c in range(NCORES):
        b, g = divmod(c, 2)
        gs = g * F
        im = {
            "xq": cvt(query[b].T),
            "xk": cvt(key[b].T),
            "xv": cvt(value[b].T),
            "wq": cvt(Wq[gs:gs + F, :].T),
            "wk": cvt(Wk[gs:gs + F, :].T),
            "wv": cvt(Wv[gs:gs + F, :].T),
            "wo": cvt(Wout[:, gs:gs + F].T),
            "bqr": np.ascontiguousarray(bq[gs:gs + F].reshape(NFC, P).T),
            "bkr": np.ascontiguousarray(bk[gs:gs + F].reshape(NFC, P).T),
        }
        if use_mask:
            m = attn_mask.T.astype(np.float64).copy()
            m[key_padding_mask[b], :] = -np.inf
            im["emask"] = np.exp(m).astype(np.float32)
        in_maps.append(im)

    global LAST_NC, LAST_IN_MAPS
    LAST_NC, LAST_IN_MAPS = nc, in_maps
    res = run_bass_kernel_spmd(nc, in_maps, list(range(NCORES)))
    globals()["LAST_RES"] = res
    LAST_EXEC_NS = res.exec_time_ns
    LAST_TRACE = res.instructions_and_trace[1] if res.instructions_and_trace else None
    globals()["LAST_INSTS"] = (res.instructions_and_trace[0]
                               if res.instructions_and_trace else None)

    extra = (bv @ Wout.T + bout).astype(np.float32)
    outp = np.empty((B, T, C), np.float32)
    for b in range(B):
        outp[b] = (res.results[2 * b]["out"].astype(np.float32)
                   + res.results[2 * b + 1]["out"].astype(np.float32)
                   + extra)
    return outp


# revision 49
# speedup vs baseline: 1.1091x; 1.1091x over previous
"""Trainium2 Bass kernel for a dense fp32 MultiHeadAttention layer.

Problem (hardcoded): B=4, T=S=2048, C=1024, 16 heads x 64 dims, fp32.
  q = query @ Wq.T + bq ; k,v likewise
  scores = (q k^T) * D**-0.5 + attn_mask + padding_mask
  out = softmax(scores) @ v -> reshape -> @ Wout.T + bout

Sharding over 8 NeuronCores: core c = (batch b = c//2, head-group g = c%2).
Each core handles one batch and 8 of the 16 heads:
  - column-parallel q/k/v projections (512-dim slice of the projections)
  - attention for its 8 heads (full T x S, on-chip scores)
  - row-parallel out_proj producing a partial (T, C) output
Host sums the two partials per batch and adds the bias terms
(bout + bv @ Wout.T, which commutes with softmax since sum(weights)=1).

v2 layout/schedule (single fused phase, bf16 matmul operands):
  - all matmul operands are bf16 (halves SBUF/DMA, enables FWL weight
    loads); PSUM accumulation and softmax denominators stay fp32.
  - schedule is software-pipelined so the PE never idles and the ACT
    engine (exp) starts ~8us in instead of after a separate projection
    phase: qproj(t0) -> kproj chunks interleaved with scores(pr0) ->
    vproj chunks interleaved with scores(pr1) -> rolling PV/scores with
    qproj(t+1)/scores(t+1) hoisted before PV(pr3)/out-proj(t).
  - scores computed transposed (s on partitions): psc = kT.T @ qT, so
    softmax's s-reduction rides the PV matmul via a ones column in v.
  - normalization: both heads' denominator rows -> one [2,TCH] recip,
    broadcast across partitions with a single 2-row one-hot matmul
    (e2.T @ rrow), then one DVE multiply per head pair.
"""

import os
import numpy as np
import ml_dtypes

import concourse.bass as bass
import concourse.mybir as mybir
import concourse.tile as tile
from concourse import bacc
from concourse.bass_utils import run_bass_kernel_spmd

# ---- problem constants ----
B, T, S, C = 4, 2048, 2048, 1024
H, D = 16, 64
NCORES = 8
F = 512            # per-core projection slice (8 heads x 64)
SCALE = D ** -0.5
P = 128
TCH = 512          # t-chunk (score free dim)
NTC = T // TCH     # 4
NSC = S // P       # 16 s-chunks
NFC = F // P       # 4 f-chunks per core
NCC = C // P       # 8 contraction chunks
SW = 512           # s-window for k/v projection chunks
NSW = S // SW      # 4
HW = 65            # v width per head incl. ones column

FP32 = mybir.dt.float32
FP32R = mybir.dt.float32r
DT = mybir.dt.bfloat16
NP_DT = ml_dtypes.bfloat16

LAST_EXEC_NS = None
LAST_TRACE = None
LAST_NC = None
LAST_IN_MAPS = None


def build(use_mask: bool, debug: bool = False):
    nc = bacc.Bacc("TRN2", target_bir_lowering=False, debug=False,
                   num_devices=NCORES)

    xq = nc.dram_tensor("xq", [C, T], DT, kind="ExternalInput")
    xk = nc.dram_tensor("xk", [C, S], DT, kind="ExternalInput")
    xv = nc.dram_tensor("xv", [C, S], DT, kind="ExternalInput")
    wq = nc.dram_tensor("wq", [C, F], DT, kind="ExternalInput")
    wk = nc.dram_tensor("wk", [C, F], DT, kind="ExternalInput")
    wv = nc.dram_tensor("wv", [C, F], DT, kind="ExternalInput")
    wo = nc.dram_tensor("wo", [F, C], DT, kind="ExternalInput")
    bqr = nc.dram_tensor("bqr", [P, NFC], FP32, kind="ExternalInput")
    bkr = nc.dram_tensor("bkr", [P, NFC], FP32, kind="ExternalInput")
    if use_mask:
        emask = nc.dram_tensor("emask", [S, T], FP32, kind="ExternalInput")
    out = nc.dram_tensor("out", [T, C], DT, kind="ExternalOutput")
    if debug:
        dbg_q = nc.dram_tensor("dbg_q", [P, NFC, TCH], DT,
                               kind="ExternalOutput")
        dbg_kt = nc.dram_tensor("dbg_kt", [P, NFC, TCH], DT,
                                kind="ExternalOutput")
        dbg_v = nc.dram_tensor("dbg_v", [P, 2, 8 * HW], DT,
                               kind="ExternalOutput")
        dbg_exp = nc.dram_tensor("dbg_exp", [P, 2, 2 * TCH], DT,
                                 kind="ExternalOutput")
        dbg_rr = nc.dram_tensor("dbg_rr", [1, 2 * TCH], FP32,
                                kind="ExternalOutput")
        dbg_rrb = nc.dram_tensor("dbg_rrb", [1, 2 * TCH], FP32,
                                 kind="ExternalOutput")
        dbg_attn = nc.dram_tensor("dbg_attn", [P, NFC, TCH], DT,
                                  kind="ExternalOutput")

    xq_r = xq.rearrange("(cc p) t -> p cc t", p=P)
    xk_r = xk.rearrange("(cc p) s -> p cc s", p=P)
    xv_r = xv.rearrange("(cc p) s -> p cc s", p=P)
    wq_r = wq.rearrange("(cc p) f -> p cc f", p=P)
    wk_r = wk.rearrange("(cc p) f -> p cc f", p=P)
    wv_r = wv.rearrange("(cc p) f -> p cc f", p=P)
    wo_r = wo.rearrange("(dc p) f -> p dc f", p=P)

    with tile.TileContext(nc) as tc:
        with (
            tc.tile_pool(name="const", bufs=1) as cp,
            tc.tile_pool(name="xs", bufs=3) as xsp,        # xk/xv staging
            tc.tile_pool(name="xqs", bufs=2) as xqp,       # xq staging
            tc.tile_pool(name="expp", bufs=2) as ep,       # exp double buffer
            tc.tile_pool(name="work", bufs=4) as ws,       # small tiles
            tc.tile_pool(name="ob", bufs=4) as obp,        # out bounce
            tc.tile_pool(name="psc", bufs=2, space="PSUM") as pscp,
            tc.tile_pool(name="ppv", bufs=2, space="PSUM") as ppvp,
            tc.tile_pool(name="pg", bufs=2, space="PSUM") as pgp,
        ):
            wq_sb = cp.tile([P, NCC, F], DT, tag="wq")
            wk_sb = cp.tile([P, NCC, F], DT, tag="wk")
            wv_sb = cp.tile([P, NCC, F], DT, tag="wv")
            wo_sb = cp.tile([P, NFC, C], DT, tag="wo")
            bq_sb = cp.tile([P, NFC], FP32, tag="bq")
            bk_sb = cp.tile([P, NFC], FP32, tag="bk")
            onesb_sb = cp.tile([1, D], DT, tag="onesb")
            kT_sb = cp.tile([P, NFC, S], DT, tag="kT")
            v_sb = cp.tile([P, NSC, 8 * HW], DT, tag="v")
            qT_sb = cp.tile([P, NFC, TCH], DT, tag="qT")
            attnT = cp.tile([P, NFC, TCH], DT, tag="attnT")
            rr32_sb = cp.tile([1, 2 * TCH], FP32, tag="rr32")
            rr32b_sb = cp.tile([1, 2 * TCH], FP32, tag="rr32b")
            rrow2_sb = cp.tile([1, 2 * TCH], DT, tag="rrow2")

            # ---- startup DMAs: interleave q-path (gates the first matmul)
            # with k-path chunks so kproj inputs land during qproj compute
            xq_t0 = xqp.tile([P, NCC, TCH], DT, tag="xq", name="xq_t")
            xk_t0 = xsp.tile([P, NCC, SW], DT, tag="xs", name="xk_t")
            for cc in range(NCC):
                nc.sync.dma_start(wq_sb[:, cc, :], wq_r[:, cc, :])
                nc.sync.dma_start(xq_t0[:, cc, :], xq_r[:, cc, 0:TCH])
                nc.sync.dma_start(wk_sb[:, cc, :], wk_r[:, cc, :])
                nc.sync.dma_start(xk_t0[:, cc, :], xk_r[:, cc, 0:SW])
            nc.sync.dma_start(bq_sb[:], bqr[:])
            nc.sync.dma_start(bk_sb[:], bkr[:])

            # ---- constants: ones row (bcast stationary) + ones columns in v
            one_sb = cp.tile([P, 1], FP32, tag="one")
            nc.any.memset(one_sb[:], 1.0)
            nc.vector.tensor_copy(onesb_sb[:],
                                  one_sb[0:1, 0:1].to_broadcast((1, D)))
            ones_dst = v_sb[:].rearrange("p s (h e) -> p s h e", e=HW)[:, :, :, D]
            nc.vector.tensor_copy(ones_dst,
                                  one_sb[:, 0:1].to_broadcast(ones_dst.shape))

            def qproj(tcx, xq_t):
                t0 = tcx * TCH
                for fcp in range(2):
                    psq = [pgp.tile([P, TCH], FP32, tag="pg", name="psq")
                           for _ in range(2)]
                    for cc in range(NCC):
                        for i in range(2):
                            fc = fcp * 2 + i
                            nc.tensor.matmul(
                                psq[i][:],
                                wq_sb[:, cc, fc * P:(fc + 1) * P],
                                xq_t[:, cc, :],
                                start=(cc == 0), stop=(cc == NCC - 1))
                    for i in range(2):
                        fc = fcp * 2 + i
                        nc.vector.tensor_scalar_add(
                            qT_sb[:, fc, :], psq[i][:], bq_sb[:, fc:fc + 1])

            def kproj(sw, xk_t):
                s0 = sw * SW
                for fcp in range(2):
                    psk = [pgp.tile([P, SW], FP32, tag="pg", name="psk")
                           for _ in range(2)]
                    for cc in range(NCC):
                        for i in range(2):
                            fc = fcp * 2 + i
                            nc.tensor.matmul(
                                psk[i][:],
                                wk_sb[:, cc, fc * P:(fc + 1) * P],
                                xk_t[:, cc, :],
                                start=(cc == 0), stop=(cc == NCC - 1))
                    for i in range(2):
                        fc = fcp * 2 + i
                        nc.vector.tensor_scalar_add(
                            kT_sb[:, fc, s0:s0 + SW],
                            psk[i][:], bk_sb[:, fc:fc + 1])

            def vproj(sw, xv_t):
                for ssp in range(2):
                    psv = [pgp.tile([P, F], FP32, tag="pg", name="psv")
                           for _ in range(2)]
                    for cc in range(NCC):
                        for i in range(2):
                            ss = ssp * 2 + i
                            nc.tensor.matmul(
                                psv[i][:],
                                xv_t[:, cc, ss * P:(ss + 1) * P],
                                wv_sb[:, cc, :],
                                start=(cc == 0), stop=(cc == NCC - 1))
                    for i in range(2):
                        sc = sw * 4 + ssp * 2 + i
                        dst = v_sb[:, sc, :].rearrange(
                            "p (h e) -> p h e", e=HW)[:, :, 0:D]
                        src = psv[i][:].rearrange("p (h e) -> p h e", e=D)
                        nc.vector.tensor_copy(dst, src)

            def scores(tcx, pr, expT, sc_list):
                t0 = tcx * TCH
                for sc in sc_list:
                    psc = pscp.tile([P, 2, TCH], FP32, tag="psc", name="psc")
                    for h in range(2):
                        nc.tensor.matmul(
                            psc[:, h, :],
                            kT_sb[h * D:(h + 1) * D, pr, sc * P:(sc + 1) * P],
                            qT_sb[h * D:(h + 1) * D, pr, :],
                            start=True, stop=True)
                    nc.scalar.activation(
                        expT[:, sc, :], psc[:].rearrange("p a b -> p (a b)"),
                        mybir.ActivationFunctionType.Exp, scale=SCALE)
                    if use_mask:
                        em_t = ws.tile([P, TCH], FP32, tag="emk", name="em_t")
                        nc.sync.dma_start(
                            em_t[:], emask[sc * P:(sc + 1) * P, t0:t0 + TCH])
                        for h in range(2):
                            nc.vector.tensor_mul(
                                expT[:, sc, h * TCH:(h + 1) * TCH],
                                expT[:, sc, h * TCH:(h + 1) * TCH],
                                em_t[:])

            def pv(pr, expT, dump=False):
                """PV accumulation + denominator recip; returns pbc-deferred
                closure (bcast matmul + final multiply) to emit later."""
                ppvs = [ppvp.tile([HW, TCH], FP32, tag="ppv", name="ppv")
                        for _ in range(2)]
                for sc in range(NSC):
                    for h in range(2):
                        hh = pr * 2 + h
                        nc.tensor.matmul(
                            ppvs[h][:],
                            v_sb[:, sc, hh * HW:(hh + 1) * HW],
                            expT[:, sc, h * TCH:(h + 1) * TCH],
                            start=(sc == 0), stop=(sc == NSC - 1))
                # denominators (PSUM row D) -> row 0 side by side -> one
                # recip at partition 0 (reciprocal_approx_fast misbehaves at
                # partition offset 64) -> one bf16 convert for the bcast
                for h in range(2):
                    nc.vector.tensor_copy(
                        rr32_sb[0:1, h * TCH:(h + 1) * TCH],
                        ppvs[h][D:D + 1, :])
                nc.vector.reciprocal_approx_fast(rr32b_sb[:], rr32_sb[:])
                nc.vector.tensor_copy(rrow2_sb[:], rr32b_sb[:])
                # numerators -> attnT (releases ppv banks)
                for h in range(2):
                    nc.vector.tensor_copy(attnT[h * D:(h + 1) * D, pr, :],
                                          ppvs[h][0:D, :])
                if dump:
                    nc.sync.dma_start(dbg_exp[:], expT[:, 0:2, :])
                    nc.sync.dma_start(dbg_rr[:], rr32_sb[:])
                    nc.sync.dma_start(dbg_rrb[:], rr32b_sb[:])

                def bcast():
                    pbc = pgp.tile([P, TCH], FP32, tag="pg", name="pbc")
                    for h in range(2):
                        nc.tensor.matmul(
                            pbc[h * D:(h + 1) * D, :],
                            onesb_sb[:],
                            rrow2_sb[0:1, h * TCH:(h + 1) * TCH],
                            start=True, stop=True)
                    nc.vector.tensor_mul(attnT[:, pr, :], attnT[:, pr, :],
                                         pbc[:])
                return bcast

            def outproj(tcx):
                t0 = tcx * TCH
                last = tcx == NTC - 1
                for tw in range(TCH // P):
                    for fh in range(2):
                        po = pgp.tile([P, TCH], FP32, tag="pg", name="po")
                        for dc in range(NFC):
                            nc.tensor.matmul(
                                po[:],
                                attnT[:, dc, tw * P:(tw + 1) * P],
                                wo_sb[:, dc, fh * TCH:(fh + 1) * TCH],
                                start=(dc == 0), stop=(dc == NFC - 1))
                        ob = obp.tile([P, TCH], DT, tag="ob", name="ob")
                        # in the drain the exp stream is over: split the
                        # bounce copies across ACT and DVE to shorten it
                        if last and (tw * 2 + fh) % 2 == 1:
                            nc.scalar.copy(ob[:], po[:])
                        else:
                            nc.vector.tensor_copy(ob[:], po[:])
                        nc.sync.dma_start(
                            out[t0 + tw * P: t0 + (tw + 1) * P,
                                fh * TCH:(fh + 1) * TCH],
                            ob[:])

            # ---------------- prologue: tcx=0 ----------------
            qproj(0, xq_t0)

            exp_bufs = {}

            def new_expT(tcx, pr):
                t_ = ep.tile([P, NSC, 2 * TCH], DT, tag="expT", name="expT")
                exp_bufs[(tcx, pr)] = t_
                return t_

            # k-projection interleaved with scores(0, pr0)
            e00 = new_expT(0, 0)
            for sw in range(NSW):
                if sw == 0:
                    xk_t = xk_t0
                else:
                    xk_t = xsp.tile([P, NCC, SW], DT, tag="xs", name="xk_t")
                    for cc in range(NCC):
                        nc.sync.dma_start(xk_t[:, cc, :],
                                          xk_r[:, cc, sw * SW:(sw + 1) * SW])
                kproj(sw, xk_t)
                scores(0, 0, e00, range(4 * sw, 4 * sw + 4))
                if sw == 0:
                    for cc in range(NCC):
                        nc.sync.dma_start(wv_sb[:, cc, :], wv_r[:, cc, :])

            # v-projection interleaved with scores(0, pr1)
            e01 = new_expT(0, 1)
            for sw in range(NSW):
                xv_t = xsp.tile([P, NCC, SW], DT, tag="xs", name="xv_t")
                for cc in range(NCC):
                    nc.sync.dma_start(xv_t[:, cc, :],
                                      xv_r[:, cc, sw * SW:(sw + 1) * SW])
                vproj(sw, xv_t)
                scores(0, 1, e01, range(4 * sw, 4 * sw + 4))
                if sw == 0:
                    for dc in range(NFC):
                        nc.sync.dma_start(wo_sb[:, dc, :], wo_r[:, dc, :])

            # ---------------- rolling main loop ----------------
            xq_tiles = {0: xq_t0}
            for tcx in range(NTC):
                if tcx + 1 < NTC:
                    xq_t = xqp.tile([P, NCC, TCH], DT, tag="xq", name="xq_t")
                    xq_tiles[tcx + 1] = xq_t
                    for cc in range(NCC):
                        nc.sync.dma_start(
                            xq_t[:, cc, :],
                            xq_r[:, cc, (tcx + 1) * TCH:(tcx + 2) * TCH])

                bc0 = pv(0, exp_bufs.pop((tcx, 0)), dump=(debug and tcx == 0))
                if debug and tcx == 0:
                    nc.sync.dma_start(dbg_q[:], qT_sb[:])
                    nc.sync.dma_start(dbg_kt[:], kT_sb[:, :, 0:TCH])
                    nc.sync.dma_start(dbg_v[:], v_sb[:, 0:2, :])
                scores(tcx, 2, new_expT(tcx, 2), range(NSC))
                bc0()
                bc1 = pv(1, exp_bufs.pop((tcx, 1)))
                scores(tcx, 3, new_expT(tcx, 3), range(NSC))
                bc1()
                bc2 = pv(2, exp_bufs.pop((tcx, 2)))
                if tcx + 1 < NTC:
                    qproj(tcx + 1, xq_tiles.pop(tcx + 1))
                    scores(tcx + 1, 0, new_expT(tcx + 1, 0), range(NSC))
                bc2()
                bc3 = pv(3, exp_bufs.pop((tcx, 3)))
                if tcx + 1 < NTC:
                    scores(tcx + 1, 1, new_expT(tcx + 1, 1), range(NSC))
                bc3()
                if debug and tcx == 0:
                    nc.sync.dma_start(dbg_attn[:], attnT[:])
                outproj(tcx)

    nc.compile()
    return nc


_CACHE = {}


def _get(use_mask: bool):
    dbg = bool(int(os.environ.get("MHA_DEBUG", "0")))
    key = (use_mask, dbg)
    if key not in _CACHE:
        _CACHE[key] = build(use_mask, debug=dbg)
    return _CACHE[key]


def kernel(query, key, value, attn_mask, key_padding_mask,
           Wq, bq, Wk, bk, Wv, bv, Wout, bout):
    global LAST_EXEC_NS, LAST_TRACE
    query = np.asarray(query, np.float32)
    key = np.asarray(key, np.float32)
    value = np.asarray(value, np.float32)
    attn_mask = np.asarray(attn_mask, np.float32)
    key_padding_mask = np.asarray(key_padding_mask)
    Wq, bq = np.asarray(Wq, np.float32), np.asarray(bq, np.float32)
    Wk, bk = np.asarray(Wk, np.float32), np.asarray(bk, np.float32)
    Wv, bv = np.asarray(Wv, np.float32), np.asarray(bv, np.float32)
    Wout, bout = np.asarray(Wout, np.float32), np.asarray(bout, np.float32)

    use_mask = bool(np.any(attn_mask)) or bool(np.any(key_padding_mask))
    nc = _get(use_mask)

    def cvt(a):
        return np.ascontiguousarray(a).astype(NP_DT)

    in_maps = []
    for c in range(NCORES):
        b, g = divmod(c, 2)
        gs = g * F
        im = {
            "xq": cvt(query[b].T),
            "xk": cvt(key[b].T),
            "xv": cvt(value[b].T),
            "wq": cvt(Wq[gs:gs + F, :].T),
            "wk": cvt(Wk[gs:gs + F, :].T),
            "wv": cvt(Wv[gs:gs + F, :].T),
            "wo": cvt(Wout[:, gs:gs + F].T),
            "bqr": np.ascontiguousarray(bq[gs:gs + F].reshape(NFC, P).T),
            "bkr": np.ascontiguousarray(bk[gs:gs + F].reshape(NFC, P).T),
        }
        if use_mask:
            m = attn_mask.T.astype(np.float64).copy()
            m[key_padding_mask[b], :] = -np.inf
            im["emask"] = np.exp(m).astype(np.float32)
        in_maps.append(im)

    global LAST_NC, LAST_IN_MAPS
    LAST_NC, LAST_IN_MAPS = nc, in_maps
    res = run_bass_kernel_spmd(nc, in_maps, list(range(NCORES)))
    globals()["LAST_RES"] = res
    LAST_EXEC_NS = res.exec_time_ns
    LAST_TRACE = res.instructions_and_trace[1] if res.instructions_and_trace else None
    globals()["LAST_INSTS"] = (res.instructions_and_trace[0]
                               if res.instructions_and_trace else None)

    extra = (bv @ Wout.T + bout).astype(np.float32)
    outp = np.empty((B, T, C), np.float32)
    for b in range(B):
        outp[b] = (res.results[2 * b]["out"].astype(np.float32)
                   + res.results[2 * b + 1]["out"].astype(np.float32)
                   + extra)
    return outp


# revision 59
# speedup vs baseline: 1.1265x; 1.0157x over previous
"""Trainium2 Bass kernel for a dense fp32 MultiHeadAttention layer.

Problem (hardcoded): B=4, T=S=2048, C=1024, 16 heads x 64 dims, fp32.
  q = query @ Wq.T + bq ; k,v likewise
  scores = (q k^T) * D**-0.5 + attn_mask + padding_mask
  out = softmax(scores) @ v -> reshape -> @ Wout.T + bout

Sharding over 8 NeuronCores: core c = (batch b = c//2, head-group g = c%2).
Each core handles one batch and 8 of the 16 heads:
  - column-parallel q/k/v projections (512-dim slice of the projections)
  - attention for its 8 heads (full T x S, on-chip scores)
  - row-parallel out_proj producing a partial (T, C) output
Host sums the two partials per batch and adds the bias terms
(bout + bv @ Wout.T, which commutes with softmax since sum(weights)=1).

v2 layout/schedule (single fused phase, bf16 matmul operands):
  - all matmul operands are bf16 (halves SBUF/DMA, enables FWL weight
    loads); PSUM accumulation and softmax denominators stay fp32.
  - schedule is software-pipelined so the PE never idles and the ACT
    engine (exp) starts ~8us in instead of after a separate projection
    phase: qproj(t0) -> kproj chunks interleaved with scores(pr0) ->
    vproj chunks interleaved with scores(pr1) -> rolling PV/scores with
    qproj(t+1)/scores(t+1) hoisted before PV(pr3)/out-proj(t).
  - scores computed transposed (s on partitions): psc = kT.T @ qT, so
    softmax's s-reduction rides the PV matmul via a ones column in v.
  - normalization: both heads' denominator rows -> one [2,TCH] recip,
    broadcast across partitions with a single 2-row one-hot matmul
    (e2.T @ rrow), then one DVE multiply per head pair.
"""

import os
import numpy as np
import ml_dtypes

import concourse.bass as bass
import concourse.mybir as mybir
import concourse.tile as tile
from concourse import bacc
from concourse.bass_utils import run_bass_kernel_spmd

# ---- problem constants ----
B, T, S, C = 4, 2048, 2048, 1024
H, D = 16, 64
NCORES = 8
F = 512            # per-core projection slice (8 heads x 64)
SCALE = D ** -0.5
P = 128
TCH = 512          # t-chunk (score free dim)
NTC = T // TCH     # 4
NSC = S // P       # 16 s-chunks
NFC = F // P       # 4 f-chunks per core
NCC = C // P       # 8 contraction chunks
SW = 512           # s-window for k/v projection chunks
NSW = S // SW      # 4
HW = 65            # v width per head incl. ones column

FP32 = mybir.dt.float32
FP32R = mybir.dt.float32r
DT = mybir.dt.bfloat16
NP_DT = ml_dtypes.bfloat16

LAST_EXEC_NS = None
LAST_TRACE = None
LAST_NC = None
LAST_IN_MAPS = None


def build(use_mask: bool, debug: bool = False):
    nc = bacc.Bacc("TRN2", target_bir_lowering=False, debug=False,
                   num_devices=NCORES)

    xq = nc.dram_tensor("xq", [C, T], DT, kind="ExternalInput")
    xk = nc.dram_tensor("xk", [C, S], DT, kind="ExternalInput")
    xv = nc.dram_tensor("xv", [C, S], DT, kind="ExternalInput")
    wq = nc.dram_tensor("wq", [C, F], DT, kind="ExternalInput")
    wk = nc.dram_tensor("wk", [C, F], DT, kind="ExternalInput")
    wv = nc.dram_tensor("wv", [C, F], DT, kind="ExternalInput")
    wo = nc.dram_tensor("wo", [F, C], DT, kind="ExternalInput")
    bqr = nc.dram_tensor("bqr", [P, NFC], FP32, kind="ExternalInput")
    bkr = nc.dram_tensor("bkr", [P, NFC], FP32, kind="ExternalInput")
    e2m = nc.dram_tensor("e2m", [P, P], DT, kind="ExternalInput")
    if use_mask:
        emask = nc.dram_tensor("emask", [S, T], FP32, kind="ExternalInput")
    out = nc.dram_tensor("out", [T, C], DT, kind="ExternalOutput")
    if debug:
        dbg_q = nc.dram_tensor("dbg_q", [P, NFC, TCH], DT,
                               kind="ExternalOutput")
        dbg_kt = nc.dram_tensor("dbg_kt", [P, NFC, TCH], DT,
                                kind="ExternalOutput")
        dbg_v = nc.dram_tensor("dbg_v", [P, 2, 8 * HW], DT,
                               kind="ExternalOutput")
        dbg_exp = nc.dram_tensor("dbg_exp", [P, 2, 2 * TCH], DT,
                                 kind="ExternalOutput")
        dbg_rr = nc.dram_tensor("dbg_rr", [1, 2 * TCH], FP32,
                                kind="ExternalOutput")
        dbg_rrb = nc.dram_tensor("dbg_rrb", [1, 2 * TCH], FP32,
                                 kind="ExternalOutput")
        dbg_attn = nc.dram_tensor("dbg_attn", [P, NFC, TCH], DT,
                                  kind="ExternalOutput")

    xq_r = xq.rearrange("(cc p) t -> p cc t", p=P)
    xk_r = xk.rearrange("(cc p) s -> p cc s", p=P)
    xv_r = xv.rearrange("(cc p) s -> p cc s", p=P)
    wq_r = wq.rearrange("(cc p) f -> p cc f", p=P)
    wk_r = wk.rearrange("(cc p) f -> p cc f", p=P)
    wv_r = wv.rearrange("(cc p) f -> p cc f", p=P)
    wo_r = wo.rearrange("(dc p) f -> p dc f", p=P)

    with tile.TileContext(nc) as tc:
        with (
            tc.tile_pool(name="const", bufs=1) as cp,
            tc.tile_pool(name="xs", bufs=3) as xsp,        # xk/xv staging
            tc.tile_pool(name="xqs", bufs=2) as xqp,       # xq staging
            tc.tile_pool(name="expp", bufs=2) as ep,       # exp double buffer
            tc.tile_pool(name="work", bufs=4) as ws,       # small tiles
            tc.tile_pool(name="ob", bufs=4) as obp,        # out bounce
            tc.tile_pool(name="psc", bufs=2, space="PSUM") as pscp,
            tc.tile_pool(name="ppv", bufs=2, space="PSUM") as ppvp,
            tc.tile_pool(name="pg", bufs=2, space="PSUM") as pgp,
        ):
            wq_sb = cp.tile([P, NCC, F], DT, tag="wq")
            wk_sb = cp.tile([P, NCC, F], DT, tag="wk")
            wv_sb = cp.tile([P, NCC, F], DT, tag="wv")
            wo_sb = cp.tile([P, NFC, C], DT, tag="wo")
            bq_sb = cp.tile([P, NFC], FP32, tag="bq")
            bk_sb = cp.tile([P, NFC], FP32, tag="bk")
            e2_sb = cp.tile([P, P], DT, tag="e2")
            kT_sb = cp.tile([P, NFC, S], DT, tag="kT")
            v_sb = cp.tile([P, NSC, 8 * HW], DT, tag="v")
            qT_sb = cp.tile([P, NFC, TCH], DT, tag="qT")
            attnT = cp.tile([P, NFC, TCH], DT, tag="attnT")
            rr32_sb = cp.tile([1, 2 * TCH], FP32, tag="rr32")
            rr32b_sb = cp.tile([1, 2 * TCH], FP32, tag="rr32b")
            rrow_sb = cp.tile([P, TCH], DT, tag="rrow")

            # ---- startup DMAs: interleave q-path (gates the first matmul)
            # with k-path chunks so kproj inputs land during qproj compute
            xq_t0 = xqp.tile([P, NCC, TCH], DT, tag="xq", name="xq_t")
            for cc in range(NCC):
                nc.sync.dma_start(wq_sb[:, cc, :], wq_r[:, cc, :])
                nc.sync.dma_start(xq_t0[:, cc, :], xq_r[:, cc, 0:TCH])
                nc.sync.dma_start(wk_sb[:, cc, :], wk_r[:, cc, :])
            nc.sync.dma_start(bq_sb[:], bqr[:])
            nc.sync.dma_start(bk_sb[:], bkr[:])

            # ---- constants: e2 one-hot rows {0,64} (host-shipped; zero
            # elsewhere so junk rrow partitions are nullified), rrow zeros,
            # ones columns in v
            one_sb = cp.tile([P, 1], FP32, tag="one")
            zero_sb = cp.tile([P, 1], FP32, tag="zero")
            nc.any.memset(one_sb[:], 1.0)
            nc.any.memset(zero_sb[:], 0.0)
            nc.sync.dma_start(e2_sb[:], e2m[:])
            nc.vector.tensor_copy(rrow_sb[:],
                                  zero_sb[:, 0:1].to_broadcast(rrow_sb.shape))
            ones_dst = v_sb[:].rearrange("p s (h e) -> p s h e", e=HW)[:, :, :, D]
            nc.vector.tensor_copy(ones_dst,
                                  one_sb[:, 0:1].to_broadcast(ones_dst.shape))

            def qproj(tcx, xq_t):
                t0 = tcx * TCH
                for fcp in range(2):
                    psq = [pgp.tile([P, TCH], FP32, tag="pg", name="psq")
                           for _ in range(2)]
                    for cc in range(NCC):
                        for i in range(2):
                            fc = fcp * 2 + i
                            nc.tensor.matmul(
                                psq[i][:],
                                wq_sb[:, cc, fc * P:(fc + 1) * P],
                                xq_t[:, cc, :],
                                start=(cc == 0), stop=(cc == NCC - 1))
                    for i in range(2):
                        fc = fcp * 2 + i
                        nc.vector.tensor_scalar_add(
                            qT_sb[:, fc, :], psq[i][:], bq_sb[:, fc:fc + 1])

            def kproj(sw, xk_t):
                s0 = sw * SW
                for fcp in range(2):
                    psk = [pgp.tile([P, SW], FP32, tag="pg", name="psk")
                           for _ in range(2)]
                    for cc in range(NCC):
                        for i in range(2):
                            fc = fcp * 2 + i
                            nc.tensor.matmul(
                                psk[i][:],
                                wk_sb[:, cc, fc * P:(fc + 1) * P],
                                xk_t[:, cc, :],
                                start=(cc == 0), stop=(cc == NCC - 1))
                    for i in range(2):
                        fc = fcp * 2 + i
                        nc.vector.tensor_scalar_add(
                            kT_sb[:, fc, s0:s0 + SW],
                            psk[i][:], bk_sb[:, fc:fc + 1])

            def vproj(sw, xv_t):
                for ssp in range(2):
                    psv = [pgp.tile([P, F], FP32, tag="pg", name="psv")
                           for _ in range(2)]
                    for cc in range(NCC):
                        for i in range(2):
                            ss = ssp * 2 + i
                            nc.tensor.matmul(
                                psv[i][:],
                                xv_t[:, cc, ss * P:(ss + 1) * P],
                                wv_sb[:, cc, :],
                                start=(cc == 0), stop=(cc == NCC - 1))
                    for i in range(2):
                        sc = sw * 4 + ssp * 2 + i
                        dst = v_sb[:, sc, :].rearrange(
                            "p (h e) -> p h e", e=HW)[:, :, 0:D]
                        src = psv[i][:].rearrange("p (h e) -> p h e", e=D)
                        nc.vector.tensor_copy(dst, src)

            def scores(tcx, pr, expT, sc_list):
                t0 = tcx * TCH
                for sc in sc_list:
                    psc = pscp.tile([P, 2, TCH], FP32, tag="psc", name="psc")
                    for h in range(2):
                        nc.tensor.matmul(
                            psc[:, h, :],
                            kT_sb[h * D:(h + 1) * D, pr, sc * P:(sc + 1) * P],
                            qT_sb[h * D:(h + 1) * D, pr, :],
                            start=True, stop=True)
                    nc.scalar.activation(
                        expT[:, sc, :], psc[:].rearrange("p a b -> p (a b)"),
                        mybir.ActivationFunctionType.Exp, scale=SCALE)
                    if use_mask:
                        em_t = ws.tile([P, TCH], FP32, tag="emk", name="em_t")
                        nc.sync.dma_start(
                            em_t[:], emask[sc * P:(sc + 1) * P, t0:t0 + TCH])
                        for h in range(2):
                            nc.vector.tensor_mul(
                                expT[:, sc, h * TCH:(h + 1) * TCH],
                                expT[:, sc, h * TCH:(h + 1) * TCH],
                                em_t[:])

            def pv(pr, expT, dump=False):
                """PV accumulation + denominator recip; returns pbc-deferred
                closure (bcast matmul + final multiply) to emit later."""
                ppvs = [ppvp.tile([HW, TCH], FP32, tag="ppv", name="ppv")
                        for _ in range(2)]
                for sc in range(NSC):
                    for h in range(2):
                        hh = pr * 2 + h
                        nc.tensor.matmul(
                            ppvs[h][:],
                            v_sb[:, sc, hh * HW:(hh + 1) * HW],
                            expT[:, sc, h * TCH:(h + 1) * TCH],
                            start=(sc == 0), stop=(sc == NSC - 1))
                # denominators (PSUM row D) -> row 0 side by side -> one
                # recip at partition 0 (reciprocal_approx_fast misbehaves at
                # partition offset 64) -> one bf16 convert for the bcast
                for h in range(2):
                    nc.vector.tensor_copy(
                        rr32_sb[0:1, h * TCH:(h + 1) * TCH],
                        ppvs[h][D:D + 1, :])
                nc.vector.reciprocal_approx_fast(rr32b_sb[:], rr32_sb[:])
                for h in range(2):
                    nc.vector.tensor_copy(rrow_sb[h * D:h * D + 1, :],
                                          rr32b_sb[0:1, h * TCH:(h + 1) * TCH])
                # numerators -> attnT (releases ppv banks)
                for h in range(2):
                    nc.vector.tensor_copy(attnT[h * D:(h + 1) * D, pr, :],
                                          ppvs[h][0:D, :])
                if dump:
                    nc.sync.dma_start(dbg_exp[:], expT[:, 0:2, :])
                    nc.sync.dma_start(dbg_rr[:], rr32_sb[:])
                    nc.sync.dma_start(dbg_rrb[:], rr32b_sb[:])

                def bcast():
                    pbc = pgp.tile([P, TCH], FP32, tag="pg", name="pbc")
                    nc.tensor.matmul(pbc[:], e2_sb[:], rrow_sb[:],
                                     start=True, stop=True)
                    nc.vector.tensor_mul(attnT[:, pr, :], attnT[:, pr, :],
                                         pbc[:])
                return bcast

            def outproj(tcx):
                t0 = tcx * TCH
                last = tcx == NTC - 1
                for tw in range(TCH // P):
                    for fh in range(2):
                        po = pgp.tile([P, TCH], FP32, tag="pg", name="po")
                        for dc in range(NFC):
                            nc.tensor.matmul(
                                po[:],
                                attnT[:, dc, tw * P:(tw + 1) * P],
                                wo_sb[:, dc, fh * TCH:(fh + 1) * TCH],
                                start=(dc == 0), stop=(dc == NFC - 1))
                        ob = obp.tile([P, TCH], DT, tag="ob", name="ob")
                        # in the drain the exp stream is over: split the
                        # bounce copies across ACT and DVE to shorten it
                        if last and fh == 1:
                            nc.scalar.copy(ob[:], po[:])
                        else:
                            nc.vector.tensor_copy(ob[:], po[:])
                        nc.sync.dma_start(
                            out[t0 + tw * P: t0 + (tw + 1) * P,
                                fh * TCH:(fh + 1) * TCH],
                            ob[:])

            # ---------------- prologue: tcx=0 ----------------
            qproj(0, xq_t0)

            exp_bufs = {}

            def new_expT(tcx, pr):
                t_ = ep.tile([P, NSC, 2 * TCH], DT, tag="expT", name="expT")
                exp_bufs[(tcx, pr)] = t_
                return t_

            # k-projection interleaved with scores(0, pr0)
            e00 = new_expT(0, 0)
            for sw in range(NSW):
                xk_t = xsp.tile([P, NCC, SW], DT, tag="xs", name="xk_t")
                for cc in range(NCC):
                    nc.sync.dma_start(xk_t[:, cc, :],
                                      xk_r[:, cc, sw * SW:(sw + 1) * SW])
                kproj(sw, xk_t)
                scores(0, 0, e00, range(4 * sw, 4 * sw + 4))
                if sw == 0:
                    for cc in range(NCC):
                        nc.sync.dma_start(wv_sb[:, cc, :], wv_r[:, cc, :])

            # v-projection interleaved with scores(0, pr1)
            e01 = new_expT(0, 1)
            for sw in range(NSW):
                xv_t = xsp.tile([P, NCC, SW], DT, tag="xs", name="xv_t")
                for cc in range(NCC):
                    nc.sync.dma_start(xv_t[:, cc, :],
                                      xv_r[:, cc, sw * SW:(sw + 1) * SW])
                vproj(sw, xv_t)
                scores(0, 1, e01, range(4 * sw, 4 * sw + 4))
                if sw == 0:
                    for dc in range(NFC):
                        nc.sync.dma_start(wo_sb[:, dc, :], wo_r[:, dc, :])

            # ---------------- rolling main loop ----------------
            xq_tiles = {0: xq_t0}
            for tcx in range(NTC):
                if tcx + 1 < NTC:
                    xq_t = xqp.tile([P, NCC, TCH], DT, tag="xq", name="xq_t")
                    xq_tiles[tcx + 1] = xq_t
                    for cc in range(NCC):
                        nc.sync.dma_start(
                            xq_t[:, cc, :],
                            xq_r[:, cc, (tcx + 1) * TCH:(tcx + 2) * TCH])

                bc0 = pv(0, exp_bufs.pop((tcx, 0)), dump=(debug and tcx == 0))
                if debug and tcx == 0:
                    nc.sync.dma_start(dbg_q[:], qT_sb[:])
                    nc.sync.dma_start(dbg_kt[:], kT_sb[:, :, 0:TCH])
                    nc.sync.dma_start(dbg_v[:], v_sb[:, 0:2, :])
                scores(tcx, 2, new_expT(tcx, 2), range(NSC))
                bc0()
                bc1 = pv(1, exp_bufs.pop((tcx, 1)))
                scores(tcx, 3, new_expT(tcx, 3), range(NSC))
                bc1()
                bc2 = pv(2, exp_bufs.pop((tcx, 2)))
                if tcx + 1 < NTC:
                    qproj(tcx + 1, xq_tiles.pop(tcx + 1))
                    scores(tcx + 1, 0, new_expT(tcx + 1, 0), range(NSC))
                bc2()
                bc3 = pv(3, exp_bufs.pop((tcx, 3)))
                if tcx + 1 < NTC:
                    scores(tcx + 1, 1, new_expT(tcx + 1, 1), range(NSC))
                bc3()
                if debug and tcx == 0:
                    nc.sync.dma_start(dbg_attn[:], attnT[:])
                outproj(tcx)

    nc.compile()
    return nc


def _e2_host():
    e2 = np.zeros((P, P), np.float32)
    e2[0, 0:D] = 1.0
    e2[D, D:P] = 1.0
    return e2.astype(NP_DT)


_CACHE = {}


def _get(use_mask: bool):
    dbg = bool(int(os.environ.get("MHA_DEBUG", "0")))
    key = (use_mask, dbg)
    if key not in _CACHE:
        _CACHE[key] = build(use_mask, debug=dbg)
    return _CACHE[key]


def kernel(query, key, value, attn_mask, key_padding_mask,
           Wq, bq, Wk, bk, Wv, bv, Wout, bout):
    global LAST_EXEC_NS, LAST_TRACE
    query = np.asarray(query, np.float32)
    key = np.asarray(key, np.float32)
    value = np.asarray(value, np.float32)
    attn_mask = np.asarray(attn_mask, np.float32)
    key_padding_mask = np.asarray(key_padding_mask)
    Wq, bq = np.asarray(Wq, np.float32), np.asarray(bq, np.float32)
    Wk, bk = np.asarray(Wk, np.float32), np.asarray(bk, np.float32)
    Wv, bv = np.asarray(Wv, np.float32), np.asarray(bv, np.float32)
    Wout, bout = np.asarray(Wout, np.float32), np.asarray(bout, np.float32)

    use_mask = bool(np.any(attn_mask)) or bool(np.any(key_padding_mask))
    nc = _get(use_mask)

    def cvt(a):
        return np.ascontiguousarray(a).astype(NP_DT)

    in_maps = []
    for c in range(NCORES):
        b, g = divmod(c, 2)
        gs = g * F
        im = {
            "xq": cvt(query[b].T),
            "xk": cvt(key[b].T),
            "xv": cvt(value[b].T),
            "wq": cvt(Wq[gs:gs + F, :].T),
            "wk": cvt(Wk[gs:gs + F, :].T),
            "wv": cvt(Wv[gs:gs + F, :].T),
            "wo": cvt(Wout[:, gs:gs + F].T),
            "bqr": np.ascontiguousarray(bq[gs:gs + F].reshape(NFC, P).T),
            "bkr": np.ascontiguousarray(bk[gs:gs + F].reshape(NFC, P).T),
            "e2m": _e2_host(),
        }
        if use_mask:
            m = attn_mask.T.astype(np.float64).copy()
            m[key_padding_mask[b], :] = -np.inf
            im["emask"] = np.exp(m).astype(np.float32)
        in_maps.append(im)

    global LAST_NC, LAST_IN_MAPS
    LAST_NC, LAST_IN_MAPS = nc, in_maps
    res = run_bass_kernel_spmd(nc, in_maps, list(range(NCORES)))
    globals()["LAST_RES"] = res
    LAST_EXEC_NS = res.exec_time_ns
    LAST_TRACE = res.instructions_and_trace[1] if res.instructions_and_trace else None
    globals()["LAST_INSTS"] = (res.instructions_and_trace[0]
                               if res.instructions_and_trace else None)

    extra = (bv @ Wout.T + bout).astype(np.float32)
    outp = np.empty((B, T, C), np.float32)
    for b in range(B):
        outp[b] = (res.results[2 * b]["out"].astype(np.float32)
                   + res.results[2 * b + 1]["out"].astype(np.float32)
                   + extra)
    return outp
